# revision 1
# baseline (speedup 1.0000x reference)
"""nn_CEBlock Trainium2 kernel — 8-core SPMD, zero-collective query-split.

Sharding: core (b, r) with b = batch (2), r = query-quarter (4).  Each core
receives x[b]^T rolled by (r*576 - 48) tokens so its 576 output tokens are at
positions 48:624 of the 2304-token window.  It computes LN1 + full k/v (all 8
heads, all tokens), q/attention for tokens 0:672 (incl. 48-token conv halo on
each side), proj + residual, LN2, fc1 -> depthwise 3x3 conv -> gelu -> fc2,
adapter, and writes out^T [512, 576] f32.  Host assembles the full output.

All data on-chip is C-major ([channel, token]); v is token-major for the AV
matmul.  Matmuls run in fp32r (stats/qkv) and fp16 (attention/MLP).
"""
import sys

sys.path.insert(0, "/opt/trn_rl_repo")

from contextlib import ExitStack

import numpy as np

import concourse.bass as bass  # noqa: F401
import concourse.tile as tile
from concourse import bacc, bass_utils, mybir

F32 = mybir.dt.float32
F32R = mybir.dt.float32r
F16 = mybir.dt.float16
AF = mybir.ActivationFunctionType

P = 128
C = 512
NTOK = 2304
QE = 672          # extended query window (576 + 2*48 halo)
QO = 576          # output tokens per core
HALO = 48
HEADS = 8
HD = 64
CM = 2048
CA = 128
NCT = C // P      # 4 channel tiles
NKT = NTOK // P   # 18 token tiles
NMT = CM // P     # 16 hidden tiles
EPS = 1e-5

# token chunking for the full 2304 range (all >=256 for fp32r speed)
CH2304 = [(i * 512, min(512, NTOK - i * 512)) for i in range((NTOK + 511) // 512)]
# query chunks for the 672 window
QCH = [(0, 336), (336, 336)]


def build(trace_scopes=False):
    nc = bacc.Bacc("TRN2", target_bir_lowering=False, debug=False, num_devices=8)

    # ---- DRAM I/O ----
    xT_d = nc.dram_tensor("xT", [C, NTOK], F32R, kind="ExternalInput").ap()
    wq_d = nc.dram_tensor("wq", [C, C], F32R, kind="ExternalInput").ap()
    wkv_d = nc.dram_tensor("wkv", [C, 2 * C], F32R, kind="ExternalInput").ap()
    wproj_d = nc.dram_tensor("wproj", [C, C], F16, kind="ExternalInput").ap()
    wfc1_d = nc.dram_tensor("wfc1", [C, CM], F16, kind="ExternalInput").ap()
    wfc2_d = nc.dram_tensor("wfc2", [CM, C], F16, kind="ExternalInput").ap()
    wa1_d = nc.dram_tensor("wa1", [C, CA], F16, kind="ExternalInput").ap()
    wa2_d = nc.dram_tensor("wa2", [CA, C], F16, kind="ExternalInput").ap()
    dwdiag_d = nc.dram_tensor("dwdiag", [NMT, P, 9, P], F16, kind="ExternalInput").ap()
    dwb_d = nc.dram_tensor("dwb", [P, NMT], F32, kind="ExternalInput").ap()
    sqneg_d = nc.dram_tensor("sqneg", [1, C], F32R, kind="ExternalInput").ap()
    skvneg_d = nc.dram_tensor("skvneg", [1, 2 * C], F32R, kind="ExternalInput").ap()
    s2neg_d = nc.dram_tensor("s2neg", [1, CM], F16, kind="ExternalInput").ap()
    consts_d = nc.dram_tensor("consts", [1, 4], F32, kind="ExternalInput").ap()
    outT_d = nc.dram_tensor("outT", [C, QO], F32, kind="ExternalOutput").ap()

    with ExitStack() as ctx:
        tc = ctx.enter_context(tile.TileContext(nc))
        # weights + tiny persistent rows: live for the whole kernel
        wp = ctx.enter_context(tc.tile_pool(name="wp", bufs=1))
        # persistent-2: outlives the attention pool (x1 etc.)
        p2p = ctx.enter_context(tc.tile_pool(name="p2p", bufs=1))
        dram = ctx.enter_context(tc.tile_pool(name="dram", bufs=1, space="DRAM"))

        # ---- weights ----
        wq_t = [wp.tile([P, C], F32R, tag=f"wq{i}", name=f"wq{i}") for i in range(NCT)]
        wkv_t = [wp.tile([P, 2 * C], F32R, tag=f"wkv{i}", name=f"wkv{i}") for i in range(NCT)]
        wproj_t = [wp.tile([P, C], F16, tag=f"wp{i}", name=f"wp{i}") for i in range(NCT)]
        for i in range(NCT):
            sl = slice(i * P, (i + 1) * P)
            nc.sync.dma_start(wq_t[i], wq_d[sl, :])
            nc.sync.dma_start(wkv_t[i], wkv_d[sl, :])
            nc.sync.dma_start(wproj_t[i], wproj_d[sl, :])
        sqneg = wp.tile([1, C], F32R, tag="sqneg", name="sqneg")
        nc.sync.dma_start(sqneg, sqneg_d)
        skvneg = wp.tile([1, 2 * C], F32R, tag="skvneg", name="skvneg")
        nc.sync.dma_start(skvneg, skvneg_d)
        consts = wp.tile([1, 4], F32, tag="consts", name="consts")
        nc.sync.dma_start(consts, consts_d)

        inv512_f = wp.tile([P, 1], F32, tag="inv512f", name="inv512f")
        nc.vector.memset(inv512_f, 1.0 / C)
        inv512 = wp.tile([P, 1], F32R, tag="inv512", name="inv512")
        nc.vector.tensor_copy(inv512, inv512_f)
        ones128_f = wp.tile([1, P], F32, tag="ones128f", name="ones128f")
        nc.vector.memset(ones128_f, 1.0)
        ones128r = wp.tile([1, P], F32R, tag="ones128r", name="ones128r")
        nc.vector.tensor_copy(ones128r, ones128_f)
        ones64h = wp.tile([1, HD], F16, tag="ones64h", name="ones64h")
        nc.vector.memset(ones64h, 1.0)
        epsrow = wp.tile([1, 1], F32, tag="epsrow", name="epsrow")
        nc.vector.memset(epsrow, EPS)

        # ---- persistent-2 (outlive attention pool) ----
        x1T = [p2p.tile([P, QE], F32R, tag=f"x1T{i}", name=f"x1T{i}") for i in range(NCT)]
        x1h = [p2p.tile([P, QE], F16, tag=f"x1h{i}", name=f"x1h{i}") for i in range(NCT)]
        mu1 = p2p.tile([1, NTOK], F32R, tag="mu1", name="mu1")
        r1col = p2p.tile([P, NKT], F32, tag="r1col", name="r1col")
        dscr = dram.tile([1, NTOK], F32, tag="dscr", name="dscr")

        # ================= attention-scoped pool =================
        with tc.tile_pool(name="ap", bufs=1) as apool, \
             tc.tile_pool(name="art", bufs=1) as art:
            xt = [apool.tile([P, NTOK], F32R, tag=f"xt{i}", name=f"xt{i}") for i in range(NCT)]
            for i in range(NCT):
                nc.sync.dma_start(xt[i], xT_d[i * P:(i + 1) * P, :])
            kT = [apool.tile([P, NTOK], F16, tag=f"kT{i}", name=f"kT{i}") for i in range(NCT)]
            vsb = [apool.tile([P, HEADS, HD + 1], F16, tag=f"v{i}", name=f"v{i}") for i in range(NKT)]
            qT = [apool.tile([P, QE], F16, tag=f"qT{i}", name=f"qT{i}") for i in range(NCT)]
            oT = [apool.tile([P, QE], F16, tag=f"oT{i}", name=f"oT{i}") for i in range(NCT)]
            R1 = apool.tile([P, NTOK], F16, tag="R1", name="R1")

            # ===== Phase 1: LN1 stats (per 512-token chunk) =====
            with tc.tile_pool(name="p1", bufs=2) as p1, \
                 tc.tile_pool(name="p1r", bufs=2) as p1r, \
                 tc.tile_pool(name="ps1", bufs=2, space="PSUM") as ps1, \
                 tc.tile_pool(name="ps2", bufs=2, space="PSUM") as ps2:
                for (c0, cw) in CH2304:
                    sl = slice(c0, c0 + cw)
                    mu_ps = ps1.tile([1, 512], F32, tag="st_ps", name="mu_ps", bufs=1)
                    for i in range(NCT):
                        nc.tensor.matmul(mu_ps[:, :cw], inv512, xt[i][:, sl],
                                         start=(i == 0), stop=(i == NCT - 1))
                    mu_c = p1r.tile([1, 512], F32, tag="mu_c", name="mu_c")
                    nc.vector.tensor_copy(mu_c[:, :cw], mu_ps[:, :cw])
                    nc.vector.tensor_copy(mu1[:, sl], mu_c[:, :cw])  # -> f32r
                    sq_ps = ps1.tile([1, 512], F32, tag="st_ps", name="sq_ps", bufs=1)
                    for i in range(NCT):
                        xsq = p1.tile([P, 512], F32R, tag="xsq", name="xsq")
                        nc.scalar.activation(xsq[:, :cw], xt[i][:, sl].bitcast(F32),
                                             AF.Square)
                        nc.tensor.matmul(sq_ps[:, :cw], inv512, xsq[:, :cw],
                                         start=(i == 0), stop=(i == NCT - 1))
                    var_c = p1r.tile([1, 512], F32, tag="var_c", name="var_c")
                    nc.vector.tensor_mul(var_c[:, :cw], mu_c[:, :cw], mu_c[:, :cw])
                    nc.vector.scalar_tensor_tensor(var_c[:, :cw], var_c[:, :cw],
                                                   -1.0, sq_ps[:, :cw],
                                                   op0=mybir.AluOpType.mult,
                                                   op1=mybir.AluOpType.add)
                    nc.scalar.activation(var_c[:, :cw], var_c[:, :cw], AF.Ln,
                                         bias=epsrow, scale=1.0)
                    r1_c = p1r.tile([1, 512], F32, tag="r1_c", name="r1_c")
                    nc.scalar.activation(r1_c[:, :cw], var_c[:, :cw], AF.Exp,
                                         scale=-0.5)
                    nc.sync.dma_start(dscr[:, sl], r1_c[:, :cw])
                    r1_cr = p1r.tile([1, 512], F32R, tag="r1_cr", name="r1_cr")
                    nc.vector.tensor_copy(r1_cr[:, :cw], r1_c[:, :cw])
                    r1_ps = ps1.tile([P, 512], F32, tag="r1b_ps", name="r1_ps", bufs=1)
                    nc.tensor.matmul(r1_ps[:, :cw], ones128r, r1_cr[:, :cw],
                                     start=True, stop=True)
                    nc.vector.tensor_copy(R1[:, sl], r1_ps[:, :cw])
                nc.sync.dma_start(r1col, dscr.rearrange("o (t p) -> (o p) t", p=P))

                # ===== Phase 2: k/v/q projections (same scope: overlaps) =====
                for kt in range(NCT):
                    wcol = slice(kt * P, (kt + 1) * P)
                    for (c0, cw) in CH2304:
                        sl = slice(c0, c0 + cw)
                        kv_ps = ps2.tile([P, 512], F32, tag="kv_ps", name="kv_ps", bufs=4)
                        for i in range(NCT):
                            nc.tensor.matmul(kv_ps[:, :cw], wkv_t[i][:, wcol],
                                             xt[i][:, sl], start=(i == 0), stop=False)
                        nc.tensor.matmul(kv_ps[:, :cw], skvneg[:, wcol], mu1[:, sl],
                                         start=False, stop=True)
                        nc.vector.tensor_mul(kT[kt][:, sl], kv_ps[:, :cw], R1[:, sl])
                for tt in range(NKT):
                    tsl = slice(tt * P, (tt + 1) * P)
                    v_ps = ps2.tile([P, C], F32, tag="v_ps", name="v_ps")
                    for i in range(NCT):
                        nc.tensor.matmul(v_ps, xt[i][:, tsl], wkv_t[i][:, C:2 * C],
                                         start=(i == 0), stop=False)
                    nc.tensor.matmul(v_ps, mu1[:, tsl], skvneg[:, C:2 * C],
                                     start=False, stop=True)
                    nc.vector.tensor_scalar_mul(
                        vsb[tt][:, :, 0:HD],
                        v_ps.rearrange("p (h d) -> p h d", h=HEADS),
                        r1col[:, tt:tt + 1])
                    nc.vector.memset(vsb[tt][:, :, HD:HD + 1], 1.0)
                for qt in range(NCT):
                    wcol = slice(qt * P, (qt + 1) * P)
                    for (q0, qw) in QCH:
                        sl = slice(q0, q0 + qw)
                        q_ps = ps2.tile([P, 512], F32, tag="kv_ps", name="q_ps", bufs=4)
                        for i in range(NCT):
                            nc.tensor.matmul(q_ps[:, :qw], wq_t[i][:, wcol],
                                             xt[i][:, sl], start=(i == 0), stop=False)
                        nc.tensor.matmul(q_ps[:, :qw], sqneg[:, wcol], mu1[:, sl],
                                         start=False, stop=True)
                        nc.vector.tensor_mul(qT[qt][:, sl], q_ps[:, :qw], R1[:, sl])

            # ===== Phase 3: attention (2-head packed, merged exp) =====
            with tc.tile_pool(name="p3", bufs=4) as p3, \
                 tc.tile_pool(name="p3r", bufs=2) as p3r, \
                 tc.tile_pool(name="ps3s", bufs=2, space="PSUM") as ps3s, \
                 tc.tile_pool(name="ps3o", bufs=1, space="PSUM") as ps3o:
                ones128h = p3r.tile([P, 1], F16, tag="ones128h", name="ones128h", bufs=1)
                nc.vector.memset(ones128h, 1.0)
                for kt in range(NCT):  # head pair (2kt, 2kt+1)
                    o2 = ps3o.tile([P, 2, 512], F32, tag="o2", name="o2", bufs=1)
                    de = p3.tile([P, QE], F16, tag="de", name="de", bufs=2)
                    do = p3.tile([P, QE], F16, tag="do", name="do", bufs=2)
                    for kc in range(NKT):
                        ksl = slice(kc * P, (kc + 1) * P)
                        for par in range(2):  # even/odd head of the pair
                            hp = slice(HD * par, HD * par + HD)
                            s2 = ps3s.tile([P, 2, 512], F32, tag="s2", name="s2")
                            for qi, (q0, qw) in enumerate(QCH):
                                nc.tensor.matmul(s2[:, qi, :qw], kT[kt][hp, ksl],
                                                 qT[kt][hp, q0:q0 + qw],
                                                 start=True, stop=True)
                            xp = p3.tile([P, QE], F16, tag="xp", name="xp")
                            nc.scalar.activation(
                                xp.rearrange("p (q x) -> p q x", x=336),
                                s2[:, :, :336], AF.Exp, scale=0.125)
                            dd = de if par == 0 else do
                            if kc == 0:
                                nc.vector.tensor_copy(dd, xp)
                            else:
                                nc.vector.tensor_add(dd, dd, xp)
                            for qi, (q0, qw) in enumerate(QCH):
                                nc.tensor.matmul(
                                    o2[HD * par:HD * par + HD, qi, :qw],
                                    vsb[kc][:, 2 * kt + par, 0:HD],
                                    xp[:, q0:q0 + qw],
                                    start=(kc == 0), stop=(kc == NKT - 1))
                    # denominators -> reciprocal -> broadcast -> normalize
                    rd_ps = ps3s.tile([P, 2, 512], F32, tag="rd_ps", name="rd_ps", bufs=1)
                    for qi, (q0, qw) in enumerate(QCH):
                        nc.tensor.matmul(rd_ps[0:1, qi, :qw], ones128h,
                                         de[:, q0:q0 + qw], start=True, stop=True)
                        nc.tensor.matmul(rd_ps[32:33, qi, :qw], ones128h,
                                         do[:, q0:q0 + qw], start=True, stop=True)
                    lnd = p3r.tile([1, QE], F32, tag="lnd", name="lnd")
                    lnd2 = p3r.tile([1, QE], F32, tag="lnd2", name="lnd2")
                    nc.scalar.activation(lnd.rearrange("p (q x) -> p q x", x=336),
                                         rd_ps[0:1, :, :336], AF.Ln)
                    nc.scalar.activation(lnd2.rearrange("p (q x) -> p q x", x=336),
                                         rd_ps[32:33, :, :336], AF.Ln)
                    rde = p3r.tile([1, QE], F16, tag="rde", name="rde")
                    rdo = p3r.tile([1, QE], F16, tag="rdo", name="rdo")
                    nc.scalar.activation(rde, lnd, AF.Exp, scale=-1.0)
                    nc.scalar.activation(rdo, lnd2, AF.Exp, scale=-1.0)
                    for qi, (q0, qw) in enumerate(QCH):
                        nc.tensor.matmul(rd_ps[0:HD, qi, :qw], ones64h,
                                         rde[:, q0:q0 + qw], start=True, stop=True)
                        nc.tensor.matmul(rd_ps[HD:P, qi, :qw], ones64h,
                                         rdo[:, q0:q0 + qw], start=True, stop=True)
                    rdsb = p3.tile([P, QE], F32, tag="rdsb", name="rdsb", bufs=2)
                    nc.vector.tensor_copy(rdsb.rearrange("p (q x) -> p q x", x=336),
                                          rd_ps[:, :, :336])
                    nc.vector.tensor_mul(oT[kt].rearrange("p (q x) -> p q x", x=336),
                                         o2[:, :, :336], rdsb.rearrange(
                                             "p (q x) -> p q x", x=336))
                for (q0, qw) in QCH:
                    qsl = slice(q0, q0 + qw)
                    for co in range(NCT):
                        pj_ps = ps3s.tile([P, 2, 512], F32, tag="s2", name="pj_ps")
                        for i in range(NCT):
                            nc.tensor.matmul(pj_ps[:, 0, :qw],
                                             wproj_t[i][:, co * P:(co + 1) * P],
                                             oT[i][:, qsl], start=(i == 0),
                                             stop=(i == NCT - 1))
                        nc.vector.tensor_add(x1T[co][:, qsl], pj_ps[:, 0, :qw],
                                             xt[co][:, qsl].bitcast(F32))
                        nc.vector.tensor_copy(x1h[co][:, qsl],
                                              x1T[co][:, qsl].bitcast(F32))

        # ================= MLP-scoped pool =================
        with tc.tile_pool(name="bp", bufs=1) as bpool:
            wfc1_t = [bpool.tile([P, CM], F16, tag=f"wf1{i}", name=f"wf1{i}") for i in range(NCT)]
            wa1_t = [bpool.tile([P, CA], F16, tag=f"wa1{i}", name=f"wa1{i}") for i in range(NCT)]
            for i in range(NCT):
                sl = slice(i * P, (i + 1) * P)
                nc.sync.dma_start(wfc1_t[i], wfc1_d[sl, :])
                nc.sync.dma_start(wa1_t[i], wa1_d[sl, :])
            wfc2_t = [bpool.tile([P, C], F16, tag=f"wf2{i}", name=f"wf2{i}") for i in range(NMT)]
            for i in range(NMT):
                nc.sync.dma_start(wfc2_t[i], wfc2_d[i * P:(i + 1) * P, :])
            wa2_t = bpool.tile([CA, C], F16, tag="wa2", name="wa2")
            nc.sync.dma_start(wa2_t, wa2_d)
            dwb_t = bpool.tile([P, NMT], F32, tag="dwb", name="dwb")
            nc.sync.dma_start(dwb_t, dwb_d)
            s2neg = bpool.tile([1, CM], F16, tag="s2neg", name="s2neg")
            nc.sync.dma_start(s2neg, s2neg_d)
            h2 = [bpool.tile([P, QO], F16, tag=f"h2{i}", name=f"h2{i}") for i in range(NMT)]
            out_sb = [bpool.tile([P, QO], F32, tag=f"osb{i}", name=f"osb{i}") for i in range(NCT)]
            R2 = bpool.tile([P, QE], F32, tag="R2", name="R2")
            mu2h = bpool.tile([1, QE], F16, tag="mu2h", name="mu2h")

            # ===== Phase 4: LN2 stats =====
            with tc.tile_pool(name="p4", bufs=2) as p4, \
                 tc.tile_pool(name="p4r", bufs=1) as p4r, \
                 tc.tile_pool(name="ps4", bufs=2, space="PSUM") as ps4:
                mu2f = p4r.tile([1, QE], F32, tag="mu2f", name="mu2f")
                ex2b = p4r.tile([1, QE], F32, tag="ex2b", name="ex2b")
                for (q0, qw) in QCH:
                    sl = slice(q0, q0 + qw)
                    m_ps = ps4.tile([1, 336], F32, tag="m_ps", name="m_ps")
                    for i in range(NCT):
                        nc.tensor.matmul(m_ps[:, :qw], inv512, x1T[i][:, sl],
                                         start=(i == 0), stop=(i == NCT - 1))
                    nc.vector.tensor_copy(mu2f[:, sl], m_ps[:, :qw])
                    s_ps4 = ps4.tile([1, 336], F32, tag="s_ps4", name="s_ps4")
                    for i in range(NCT):
                        x1sq = p4.tile([P, 336], F32R, tag="x1sq", name="x1sq")
                        nc.scalar.activation(x1sq[:, :qw], x1T[i][:, sl].bitcast(F32),
                                             AF.Square)
                        nc.tensor.matmul(s_ps4[:, :qw], inv512, x1sq[:, :qw],
                                         start=(i == 0), stop=(i == NCT - 1))
                    nc.vector.tensor_copy(ex2b[:, sl], s_ps4[:, :qw])
                mu2r = p4r.tile([1, QE], F32R, tag="mu2r", name="mu2r")
                nc.vector.tensor_copy(mu2r, mu2f)
                nc.vector.tensor_copy(mu2h, mu2f)
                var2 = p4r.tile([1, QE], F32, tag="var2", name="var2")
                nc.vector.tensor_mul(var2, mu2f, mu2f)
                nc.vector.scalar_tensor_tensor(var2, var2, -1.0, ex2b,
                                               op0=mybir.AluOpType.mult,
                                               op1=mybir.AluOpType.add)
                lnv2 = p4r.tile([1, QE], F32, tag="lnv2", name="lnv2")
                nc.scalar.activation(lnv2, var2, AF.Ln, bias=epsrow, scale=1.0)
                r2row = p4r.tile([1, QE], F32, tag="r2row", name="r2row")
                nc.scalar.activation(r2row, lnv2, AF.Exp, scale=-0.5)
                nc.vector.tensor_scalar_mul(r2row[:, 0:HALO], r2row[:, 0:HALO],
                                            consts[:, 0:1])
                nc.vector.tensor_scalar_mul(r2row[:, QE - HALO:QE],
                                            r2row[:, QE - HALO:QE], consts[:, 1:2])
                r2r = p4r.tile([1, QE], F32R, tag="r2r", name="r2r")
                nc.vector.tensor_copy(r2r, r2row)
                for (q0, qw) in QCH:
                    r2_ps = ps4.tile([P, 336], F32, tag="r2_ps", name="r2_ps")
                    nc.tensor.matmul(r2_ps[:, :qw], ones128r, r2r[:, q0:q0 + qw],
                                     start=True, stop=True)
                    nc.vector.tensor_copy(R2[:, q0:q0 + qw], r2_ps[:, :qw])

            # ===== Phase 5: fc1 -> dwconv -> gelu =====
            with tc.tile_pool(name="p5", bufs=2) as p5, \
                 tc.tile_pool(name="ps5a", bufs=2, space="PSUM") as ps5a, \
                 tc.tile_pool(name="ps5b", bufs=2, space="PSUM") as ps5b:
                for m in range(NMT):
                    mcol = slice(m * P, (m + 1) * P)
                    h1p = p5.tile([P, 14, 50], F16, tag="h1p", name="h1p")
                    nc.vector.memset(h1p[:, :, 0:1], 0.0)
                    nc.vector.memset(h1p[:, :, 49:50], 0.0)
                    dwt = p5.tile([P, 9, P], F16, tag="dwt", name="dwt")
                    nc.sync.dma_start(dwt, dwdiag_d[m])
                    for half in range(2):
                        sl = slice(half * 336, half * 336 + 336)
                        f1_ps = ps5a.tile([P, 336], F32, tag="f1_ps", name="f1_ps")
                        for i in range(NCT):
                            nc.tensor.matmul(f1_ps, wfc1_t[i][:, mcol], x1h[i][:, sl],
                                             start=(i == 0), stop=False)
                        nc.tensor.matmul(f1_ps, s2neg[:, mcol], mu2h[:, sl],
                                         start=False, stop=True)
                        nc.vector.tensor_mul(
                            h1p[:, half * 7:half * 7 + 7, 1:49],
                            f1_ps.rearrange("p (r x) -> p r x", x=48),
                            R2[:, sl].rearrange("p (r x) -> p r x", x=48))
                    for half in range(2):
                        cv_ps = ps5b.tile([P, 288], F32, tag="cv_ps", name="cv_ps")
                        for s in range(9):
                            dy, dx = s // 3, s % 3
                            y0 = 6 * half + dy
                            nc.tensor.matmul(cv_ps, dwt[:, s, :],
                                             h1p[:, y0:y0 + 6, dx:dx + 48],
                                             start=(s == 0), stop=(s == 8))
                        nc.scalar.activation(h2[m][:, half * 288:half * 288 + 288],
                                             cv_ps, AF.Gelu, bias=dwb_t[:, m:m + 1],
                                             scale=1.0)

            # ===== Phase 6: adapter (x1-only, overlaps LN2) then fc2 =====
            with tc.tile_pool(name="p6", bufs=2) as p6, \
                 tc.tile_pool(name="ps6", bufs=2, space="PSUM") as ps6:
                ACH = [(0, 288), (288, 288)]
                for (q0, qw) in ACH:
                    sl = slice(HALO + q0, HALO + q0 + qw)
                    a1_ps = ps6.tile([CA, 288], F32, tag="a1_ps", name="a1_ps")
                    for i in range(NCT):
                        nc.tensor.matmul(a1_ps, wa1_t[i], x1h[i][:, sl],
                                         start=(i == 0), stop=(i == NCT - 1))
                    a1sb = p6.tile([CA, 288], F16, tag="a1sb", name="a1sb")
                    nc.scalar.activation(a1sb, a1_ps, AF.Relu)
                    for co in range(NCT):
                        a2_ps = ps6.tile([P, 288], F32, tag="a2_ps", name="a2_ps")
                        nc.tensor.matmul(a2_ps, wa2_t[:, co * P:(co + 1) * P], a1sb,
                                         start=True, stop=True)
                        nc.vector.tensor_add(
                            out_sb[co][:, q0:q0 + qw], a2_ps,
                            x1T[co][:, HALO + q0:HALO + q0 + qw].bitcast(F32))
                F2CH = [(0, 512), (512, 64)]
                for co in range(NCT):
                    ccol = slice(co * P, (co + 1) * P)
                    for (q0, qw) in F2CH:
                        f2_ps = ps6.tile([P, 512], F32, tag="f2_ps", name="f2_ps")
                        for m in range(NMT):
                            nc.tensor.matmul(f2_ps[:, :qw], wfc2_t[m][:, ccol],
                                             h2[m][:, q0:q0 + qw],
                                             start=(m == 0), stop=(m == NMT - 1))
                        nc.vector.tensor_add(out_sb[co][:, q0:q0 + qw],
                                             out_sb[co][:, q0:q0 + qw],
                                             f2_ps[:, :qw])
                for co in range(NCT):
                    nc.sync.dma_start(outT_d[co * P:(co + 1) * P, :], out_sb[co])

    nc.compile()
    return nc


# ---------------- host side ----------------

_cache = {}


def _prep_shared(inputs):
    g1 = np.asarray(inputs["g1"], np.float32)
    b1 = np.asarray(inputs["b1"], np.float32)
    g2 = np.asarray(inputs["g2"], np.float32)
    b2 = np.asarray(inputs["b2"], np.float32)
    wq = np.asarray(inputs["wq"], np.float32)
    wkv = np.asarray(inputs["wkv"], np.float32)
    wproj = np.asarray(inputs["wproj"], np.float32)
    wfc1 = np.asarray(inputs["w_fc1"], np.float32)
    wfc2 = np.asarray(inputs["w_fc2"], np.float32)
    wa1 = np.asarray(inputs["wa1"], np.float32)
    wa2 = np.asarray(inputs["wa2"], np.float32)
    dw_k = np.asarray(inputs["dw_k"], np.float32)
    for nm in ("bq", "bkv", "bproj", "b_fc1", "b_fc2", "ba1", "ba2"):
        assert not np.any(np.asarray(inputs[nm])), f"nonzero bias {nm} unsupported"
    assert not np.any(b1) and not np.any(b2), "nonzero LN bias unsupported"

    wq_f = (g1[:, None] * wq).astype(np.float32)
    wkv_f = (g1[:, None] * wkv).astype(np.float32)
    wfc1_f = (g2[:, None] * wfc1).astype(np.float32)

    k9 = dw_k[:, 0].reshape(CM, 9)
    dwdiag = np.zeros((NMT, P, 9, P), np.float16)
    for m in range(NMT):
        blk = k9[m * P:(m + 1) * P]  # [128, 9]
        for s in range(9):
            dwdiag[m, np.arange(P), s, np.arange(P)] = blk[:, s].astype(np.float16)
    dwb = np.asarray(inputs["dw_b"], np.float32).reshape(NMT, P).T.copy()

    shared = {
        "wq": wq_f,
        "wkv": wkv_f,
        "wproj": wproj.astype(np.float16),
        "wfc1": wfc1_f.astype(np.float16),
        "wfc2": wfc2.astype(np.float16),
        "wa1": wa1.astype(np.float16),
        "wa2": (0.5 * wa2).astype(np.float16),
        "dwdiag": dwdiag,
        "dwb": np.ascontiguousarray(dwb),
        "sqneg": -wq_f.sum(0, keepdims=True),
        "skvneg": -wkv_f.sum(0, keepdims=True),
        "s2neg": -wfc1_f.sum(0, keepdims=True).astype(np.float16),
    }
    return shared


def run(inputs, trace=False):
    x = np.asarray(inputs["x"], np.float32)
    B, N, Cc = x.shape
    assert (B, N, Cc) == (2, NTOK, C)
    assert int(inputs["H"]) == 48 and int(inputs["W"]) == 48

    shared = _prep_shared(inputs)
    if "nc" not in _cache:
        _cache["nc"] = build()
    nc = _cache["nc"]

    in_maps = []
    for core in range(8):
        b, r = core // 4, core % 4
        roll = r * QO - HALO
        idx = (np.arange(NTOK) + roll) % NTOK
        xTc = np.ascontiguousarray(x[b].T[:, idx])
        consts = np.array([[0.0 if r == 0 else 1.0,
                            0.0 if r == 3 else 1.0, 0.0, 0.0]], np.float32)
        m = dict(shared)
        m["xT"] = xTc
        m["consts"] = consts
        in_maps.append(m)

    res = bass_utils.run_bass_kernel_spmd(nc, in_maps, core_ids=list(range(8)),
                                          trace=trace)
    out = np.empty((B, N, C), np.float32)
    for core in range(8):
        b, r = core // 4, core % 4
        out[b, r * QO:(r + 1) * QO, :] = res.results[core]["outT"].T
    return out, res


def kernel(**inputs):
    out, _ = run(inputs, trace=False)
    return out



# revision 3
# speedup vs baseline: 1.1659x; 1.1659x over previous
"""nn_CEBlock Trainium2 kernel — 8-core SPMD, zero-collective query-split, fp8.

Sharding: core (b, r) with b = batch (2), r = query-quarter (4).  Each core
receives x[b]^T rolled by (r*576 - 48) tokens so its 576 output tokens sit at
positions 48:624 of the 2304-token window.  Full k/v over all 2304 tokens is
computed per core; q/attention/MLP run on the 672-token window (576 + halo).

Fast paths:
  - All heavy matmuls run fp8e4m3 with DoubleRow perf mode (2 contraction
    sub-tiles per matmul).  Weights are host-scaled by 16 into fp8's normal
    range; power-of-2 compensation is folded into activation scales and
    scalar_tensor_tensor immediates.
  - LayerNorm mean subtraction is folded into the weights on the host
    (column-centered W' gives x@W' == (x-mu)@W exactly).
  - Scores (contraction 64) use DoubleRow with a zero second plane on the
    rhs (q tile carries a zeroed 336-col tail).
  - Softmax denominators come free from the AV matmul via a ones-row
    appended to v (row 64 of the 65-row stationary operand).
  - Elementwise work is split across DVE / Pool(gpsimd) / Act engines.
"""
import sys

sys.path.insert(0, "/opt/trn_rl_repo")

from contextlib import ExitStack

import ml_dtypes
import numpy as np

import concourse.bass as bass  # noqa: F401
import concourse.tile as tile
from concourse import bacc, bass_utils, mybir

F32 = mybir.dt.float32
F32R = mybir.dt.float32r
F16 = mybir.dt.float16
F8 = mybir.dt.float8e4
AF = mybir.ActivationFunctionType
DR = mybir.MatmulPerfMode.DoubleRow
MUL = mybir.AluOpType.mult
ADD = mybir.AluOpType.add
SUB = mybir.AluOpType.subtract
NPF8 = ml_dtypes.float8_e4m3

P = 128
C = 512
NTOK = 2304
QE = 672          # extended query window (576 + 2*48 halo)
QO = 576
HALO = 48
HEADS = 8
HD = 64
CM = 2048
CA = 128
NCT = 4
NKT = 18
NMT = 16
EPS = 1e-5
WS = 16.0         # host weight scale (power of 2)

CH2304 = [(i * 512, min(512, NTOK - i * 512)) for i in range((NTOK + 511) // 512)]
QCH = [(0, 336), (336, 336)]
ACH = [(0, 288), (288, 288)]
F2CH = [(0, 512), (512, 64)]

# score chunks routed to DVE polynomial exp instead of ScalarE (tuned): the
# poly is (1 + u/8)^8 with u = score/8, |rel err| < 1% over +-2 sigma.
POLY_KC = frozenset()


def build(trace_scopes=False):
    nc = bacc.Bacc("TRN2", target_bir_lowering=False, debug=False, num_devices=8)

    # ---- DRAM I/O ----
    xT16_d = nc.dram_tensor("xT16", [C, NTOK], F16, kind="ExternalInput").ap()
    xT8_d = nc.dram_tensor("xT8", [2, P, 2, NTOK], F8, kind="ExternalInput").ap()
    wkv8_d = nc.dram_tensor("wkv8", [2, P, 2, 2 * C], F8, kind="ExternalInput").ap()
    wq8_d = nc.dram_tensor("wq8", [2, P, 2, C], F8, kind="ExternalInput").ap()
    wproj8_d = nc.dram_tensor("wproj8", [2, P, 2, C], F8, kind="ExternalInput").ap()
    wfc18_d = nc.dram_tensor("wfc18", [2, P, 2, CM], F8, kind="ExternalInput").ap()
    wfc28_d = nc.dram_tensor("wfc28", [8, P, 2, C], F8, kind="ExternalInput").ap()
    wa18_d = nc.dram_tensor("wa18", [2, P, 2, CA], F8, kind="ExternalInput").ap()
    wa28_d = nc.dram_tensor("wa28", [CA, 2, C], F8, kind="ExternalInput").ap()
    dwp8_d = nc.dram_tensor("dwp8", [NMT, P, 3, 2, P], F8, kind="ExternalInput").ap()
    dws8_d = nc.dram_tensor("dws8", [NMT, P, 3, P], F8, kind="ExternalInput").ap()
    consts_d = nc.dram_tensor("consts", [1, 4], F32, kind="ExternalInput").ap()
    outT_d = nc.dram_tensor("outT", [C, QO], F32, kind="ExternalOutput").ap()

    LN4 = float(np.log(4.0))

    with ExitStack() as ctx:
        tc = ctx.enter_context(tile.TileContext(nc))
        wp = ctx.enter_context(tc.tile_pool(name="wp", bufs=1))
        dram = ctx.enter_context(tc.tile_pool(name="dram", bufs=1, space="DRAM"))

        # ---- persistent SBUF ----
        xt16 = [wp.tile([P, NTOK], F16, tag=f"xt16_{i}", name=f"xt16_{i}")
                for i in range(NCT)]
        xt8 = [wp.tile([P, 2, NTOK], F8, tag=f"xt8_{j}", name=f"xt8_{j}")
               for j in range(2)]
        wkv8_t = [wp.tile([P, 2, 2 * C], F8, tag=f"wkv8_{j}", name=f"wkv8_{j}")
                  for j in range(2)]
        wq8_t = [wp.tile([P, 2, C], F8, tag=f"wq8_{j}", name=f"wq8_{j}")
                 for j in range(2)]
        wproj8_t = [wp.tile([P, 2, C], F8, tag=f"wpj8_{j}", name=f"wpj8_{j}")
                    for j in range(2)]
        wfc18_t = [wp.tile([P, 2, CM], F8, tag=f"wf18_{j}", name=f"wf18_{j}")
                   for j in range(2)]
        wfc28_t = [wp.tile([P, 2, C], F8, tag=f"wf28_{j}", name=f"wf28_{j}")
                   for j in range(8)]
        wa18_t = [wp.tile([P, 2, CA], F8, tag=f"wa18_{j}", name=f"wa18_{j}")
                  for j in range(2)]
        wa28_t = wp.tile([CA, 2, C], F8, tag="wa28", name="wa28")
        consts = wp.tile([1, 4], F32, tag="consts", name="consts")
        for i in range(NCT):
            nc.sync.dma_start(xt16[i], xT16_d[i * P:(i + 1) * P, :])
        for j in range(2):
            nc.sync.dma_start(xt8[j], xT8_d[j])
            nc.sync.dma_start(wkv8_t[j], wkv8_d[j])
            nc.sync.dma_start(wq8_t[j], wq8_d[j])
            nc.sync.dma_start(wproj8_t[j], wproj8_d[j])
            nc.sync.dma_start(wfc18_t[j], wfc18_d[j])
            nc.sync.dma_start(wa18_t[j], wa18_d[j])
        for j in range(8):
            nc.sync.dma_start(wfc28_t[j], wfc28_d[j])
        nc.sync.dma_start(wa28_t, wa28_d)
        nc.sync.dma_start(consts, consts_d)

        inv512h = wp.tile([P, 1], F16, tag="inv512h", name="inv512h")
        nc.vector.memset(inv512h, 1.0 / C)
        ones128f = wp.tile([1, P], F32, tag="ones128f", name="ones128f")
        nc.vector.memset(ones128f, 1.0)
        ones128r = wp.tile([1, P], F32R, tag="ones128r", name="ones128r")
        nc.vector.tensor_copy(ones128r, ones128f)
        ones64h = wp.tile([1, HD], F16, tag="ones64h", name="ones64h")
        nc.vector.memset(ones64h, 1.0)
        epsrow = wp.tile([1, 1], F32, tag="epsrow", name="epsrow")
        nc.vector.memset(epsrow, EPS)
        ln4row = wp.tile([1, 1], F32, tag="ln4row", name="ln4row")
        nc.vector.memset(ln4row, LN4)

        R1 = wp.tile([P, NTOK], F16, tag="R1", name="R1")
        r1row = wp.tile([1, NTOK], F32, tag="r1row", name="r1row")
        r1col = wp.tile([P, NKT], F32, tag="r1col", name="r1col")
        dscr = dram.tile([1, NTOK], F32, tag="dscr", name="dscr")

        kT8 = [wp.tile([P, 2432], F8, tag=f"kT8_{i}", name=f"kT8_{i}")
               for i in range(NCT)]
        qT8 = [wp.tile([P, 1008], F8, tag=f"qT8_{i}", name=f"qT8_{i}")
               for i in range(NCT)]
        vsb2 = [wp.tile([P, 2, HEADS, 66], F8, tag=f"v2_{i}", name=f"v2_{i}")
                for i in range(9)]
        xp2 = [wp.tile([P, 2, QE], F8, tag=f"xp2_{h}", name=f"xp2_{h}")
               for h in range(HEADS)]
        oT8 = [wp.tile([P, 2, QE], F8, tag=f"oT8_{t}", name=f"oT8_{t}")
               for t in range(2)]
        x1_16 = [wp.tile([P, QE], F16, tag=f"x1_{i}", name=f"x1_{i}")
                 for i in range(NCT)]
        x1_8 = [wp.tile([P, 2, QE], F8, tag=f"x18_{j}", name=f"x18_{j}")
                for j in range(2)]
        R2 = wp.tile([P, QE], F16, tag="R2", name="R2")
        h28 = [wp.tile([P, 2, QO], F8, tag=f"h28_{j}", name=f"h28_{j}")
               for j in range(8)]
        a1sb = wp.tile([CA, 3 * 288], F8, tag="a1sb", name="a1sb")
        out_sb = [wp.tile([P, QO], F32, tag=f"osb_{i}", name=f"osb_{i}")
                  for i in range(NCT)]

        # static zero regions
        for i in range(NCT):
            nc.vector.memset(kT8[i][:, NTOK:2432], 0.0)
            nc.vector.memset(qT8[i][:, QE:1008], 0.0)
        for t in range(9):
            nc.gpsimd.memset(vsb2[t][:, :, :, HD:HD + 1], 1.0)
        nc.vector.memset(a1sb[:, 2 * 288:3 * 288], 0.0)

        # ===== Phase 1: LN1 stats -> r1row, R1, r1col =====
        with tc.tile_pool(name="p1", bufs=2) as p1, \
             tc.tile_pool(name="p1r", bufs=2) as p1r, \
             tc.tile_pool(name="ps1", bufs=2, space="PSUM") as ps1, \
             tc.tile_pool(name="ps1b", bufs=2, space="PSUM") as ps1b:
            for (c0, cw) in CH2304:
                sl = slice(c0, c0 + cw)
                mu_ps = ps1.tile([1, 512], F32, tag="mu_ps", name="mu_ps")
                for i in range(NCT):
                    nc.tensor.matmul(mu_ps[:, :cw], inv512h, xt16[i][:, sl],
                                     start=(i == 0), stop=(i == NCT - 1))
                sq_ps = ps1.tile([1, 512], F32, tag="sq_ps", name="sq_ps")
                for i in range(NCT):
                    xsq = p1.tile([P, 512], F16, tag="xsq", name="xsq")
                    nc.vector.tensor_mul(xsq[:, :cw], xt16[i][:, sl], xt16[i][:, sl])
                    nc.tensor.matmul(sq_ps[:, :cw], inv512h, xsq[:, :cw],
                                     start=(i == 0), stop=(i == NCT - 1))
                mu_c = p1r.tile([1, 512], F32, tag="mu_c", name="mu_c")
                nc.gpsimd.tensor_copy(mu_c[:, :cw], mu_ps[:, :cw])
                mu2_c = p1r.tile([1, 512], F32, tag="mu2_c", name="mu2_c")
                nc.gpsimd.tensor_mul(mu2_c[:, :cw], mu_c[:, :cw], mu_c[:, :cw])
                var_c = p1r.tile([1, 512], F32, tag="var_c", name="var_c")
                nc.gpsimd.tensor_sub(var_c[:, :cw], sq_ps[:, :cw], mu2_c[:, :cw])
                lnv = p1r.tile([1, 512], F32, tag="lnv", name="lnv")
                nc.scalar.activation(lnv[:, :cw], var_c[:, :cw], AF.Ln,
                                     bias=epsrow, scale=1.0)
                nc.scalar.activation(r1row[:, sl], lnv[:, :cw], AF.Exp, scale=-0.5)
                nc.sync.dma_start(dscr[:, sl], r1row[:, sl])
                r1b_ps = ps1b.tile([P, 512], F32, tag="r1b", name="r1b")
                nc.tensor.matmul(r1b_ps[:, :cw], ones128r,
                                 r1row[:, sl].bitcast(F32R), start=True, stop=True)
                nc.gpsimd.tensor_copy(R1[:, sl], r1b_ps[:, :cw])
            nc.sync.dma_start(r1col, dscr.rearrange("o (t p) -> (o p) t", p=P))

        # ===== Phase 2: k / v / q projections (fp8 DR) =====
        with tc.tile_pool(name="ps2", bufs=4, space="PSUM") as ps2:
            for kt in range(NCT):
                wcol = slice(kt * P, (kt + 1) * P)
                for (c0, cw) in CH2304:
                    sl = slice(c0, c0 + cw)
                    kv_ps = ps2.tile([P, 512], F32, tag="kv_ps", name="kv_ps")
                    for j in range(2):
                        nc.tensor.matmul(kv_ps[:, :cw], wkv8_t[j][:, :, wcol],
                                         xt8[j][:, :, sl], start=(j == 0),
                                         stop=(j == 1), perf_mode=DR)
                    nc.vector.tensor_mul(kT8[kt][:, sl], kv_ps[:, :cw], R1[:, sl])
            for tt in range(NKT):
                tsl = slice(tt * P, (tt + 1) * P)
                v_ps = ps2.tile([P, 512], F32, tag="kv_ps", name="v_ps")
                for j in range(2):
                    nc.tensor.matmul(v_ps, xt8[j][:, :, tsl],
                                     wkv8_t[j][:, :, C:2 * C], start=(j == 0),
                                     stop=(j == 1), perf_mode=DR)
                nc.gpsimd.tensor_scalar_mul(
                    vsb2[tt // 2][:, tt % 2, :, 0:HD],
                    v_ps.rearrange("p (h d) -> p h d", h=HEADS),
                    r1col[:, tt:tt + 1])
            for qt in range(NCT):
                wcol = slice(qt * P, (qt + 1) * P)
                for (q0, qw) in QCH:
                    sl = slice(q0, q0 + qw)
                    q_ps = ps2.tile([P, 512], F32, tag="kv_ps", name="q_ps")
                    for j in range(2):
                        nc.tensor.matmul(q_ps[:, :qw], wq8_t[j][:, :, wcol],
                                         xt8[j][:, :, sl], start=(j == 0),
                                         stop=(j == 1), perf_mode=DR)
                    nc.vector.tensor_mul(qT8[qt][:, sl], q_ps[:, :qw], R1[:, sl])

        # ===== Phase 3: attention =====
        with tc.tile_pool(name="p3", bufs=2) as p3, \
             tc.tile_pool(name="p3r", bufs=2) as p3r, \
             tc.tile_pool(name="ps3s", bufs=2, space="PSUM") as ps3s, \
             tc.tile_pool(name="ps3o", bufs=2, space="PSUM") as ps3o:
            for ht in range(NCT):
                kv = kT8[ht].rearrange("p (a b) -> p a b", b=P)      # [P,19,128]
                qv = qT8[ht].rearrange("p (a b) -> p a b", b=336)    # [P,3,336]
                for par in range(2):
                    h = 2 * ht + par
                    hp = slice(HD * par, HD * par + HD)
                    o2 = ps3o.tile([P, 2, 512], F32, tag="o2", name="o2")
                    for kc in range(NKT):
                        s2 = ps3s.tile([P, 2, 512], F32, tag="s2", name="s2")
                        for qi in range(2):
                            rhs = (qv[hp, 0:3:2, :] if qi == 0
                                   else qv[hp, 1:3, :])
                            nc.tensor.matmul(s2[:, qi, 0:336],
                                             kv[hp, kc:kc + 2, :], rhs,
                                             start=True, stop=True, perf_mode=DR)
                        if kc in POLY_KC:
                            t1 = p3.tile([P, QE], F16, tag="pt1", name="pt1")
                            nc.vector.tensor_scalar(
                                t1.rearrange("p (a b) -> p a b", b=336),
                                s2[:, :, 0:336], 2.0 ** -14, 1.0,
                                op0=MUL, op1=ADD)
                            t2 = p3.tile([P, QE], F16, tag="pt2", name="pt2")
                            nc.vector.tensor_mul(t2, t1, t1)
                            t3 = p3.tile([P, QE], F16, tag="pt3", name="pt3")
                            nc.vector.tensor_mul(t3, t2, t2)
                            nc.vector.tensor_mul(xp2[h][:, kc % 2, :], t3, t3)
                        else:
                            nc.scalar.activation(
                                xp2[h][:, kc % 2, :].rearrange(
                                    "p (a b) -> p a b", b=336),
                                s2[:, :, 0:336], AF.Exp, scale=0.125 / 256.0)
                        if kc % 2 == 1:
                            kcp = kc // 2
                            for qi, (q0, qw) in enumerate(QCH):
                                nc.tensor.matmul(
                                    o2[0:HD + 1, qi, 0:336],
                                    vsb2[kcp][:, :, h, 0:HD + 1],
                                    xp2[h][:, :, q0:q0 + qw],
                                    start=(kcp == 0), stop=(kcp == 8),
                                    perf_mode=DR)
                    # denominators -> 4/d -> broadcast -> normalize into oT8
                    lnd = p3r.tile([1, QE], F32, tag="lnd", name="lnd")
                    nc.scalar.activation(lnd.rearrange("p (a b) -> p a b", b=336),
                                         o2[HD:HD + 1, :, 0:336], AF.Ln)
                    rde = p3r.tile([1, QE], F16, tag="rde", name="rde")
                    nc.scalar.activation(rde, lnd, AF.Exp, bias=ln4row, scale=-1.0)
                    rd_ps = ps3s.tile([P, 2, 512], F32, tag="s2", name="rd_ps")
                    for qi, (q0, qw) in enumerate(QCH):
                        nc.tensor.matmul(rd_ps[0:HD, qi, 0:336], ones64h,
                                         rde[:, q0:q0 + qw], start=True, stop=True)
                    rdsb = p3.tile([HD, QE], F32, tag="rdsb", name="rdsb")
                    nc.gpsimd.tensor_copy(rdsb.rearrange("p (a b) -> p a b", b=336),
                                          rd_ps[0:HD, :, 0:336])
                    nc.vector.tensor_mul(
                        oT8[ht // 2][hp, ht % 2, :].rearrange(
                            "p (a b) -> p a b", b=336),
                        o2[0:HD, :, 0:336],
                        rdsb.rearrange("p (a b) -> p a b", b=336))

        # ===== Phase 4: proj + residual -> x1 =====
        with tc.tile_pool(name="ps4", bufs=2, space="PSUM") as ps4:
            for (q0, qw) in QCH:
                sl = slice(q0, q0 + qw)
                for co in range(NCT):
                    pj = ps4.tile([P, 512], F32, tag="pj", name="pj")
                    for tp in range(2):
                        nc.tensor.matmul(pj[:, :qw],
                                         wproj8_t[tp][:, :, co * P:(co + 1) * P],
                                         oT8[tp][:, :, sl], start=(tp == 0),
                                         stop=(tp == 1), perf_mode=DR)
                    nc.vector.scalar_tensor_tensor(x1_16[co][:, sl], pj[:, :qw],
                                                   2.0 ** -10, xt16[co][:, sl],
                                                   op0=MUL, op1=ADD)
            for co in range(NCT):
                nc.vector.tensor_copy(x1_8[co // 2][:, co % 2, :], x1_16[co])

        # ===== Phase 5: LN2 -> R2 =====
        with tc.tile_pool(name="p5", bufs=2) as p5, \
             tc.tile_pool(name="p5r", bufs=1) as p5r, \
             tc.tile_pool(name="ps5", bufs=2, space="PSUM") as ps5:
            r2row = p5r.tile([1, QE], F32, tag="r2row", name="r2row")
            for (q0, qw) in QCH:
                sl = slice(q0, q0 + qw)
                m_ps = ps5.tile([1, 512], F32, tag="m_ps", name="m_ps")
                for i in range(NCT):
                    nc.tensor.matmul(m_ps[:, :qw], inv512h, x1_16[i][:, sl],
                                     start=(i == 0), stop=(i == NCT - 1))
                s_ps = ps5.tile([1, 512], F32, tag="s_ps", name="s_ps")
                for i in range(NCT):
                    x1sq = p5.tile([P, 336], F16, tag="x1sq", name="x1sq")
                    nc.vector.tensor_mul(x1sq[:, :qw], x1_16[i][:, sl],
                                         x1_16[i][:, sl])
                    nc.tensor.matmul(s_ps[:, :qw], inv512h, x1sq[:, :qw],
                                     start=(i == 0), stop=(i == NCT - 1))
                mu_c = p5.tile([1, 512], F32, tag="mu_c5", name="mu_c5")
                nc.gpsimd.tensor_copy(mu_c[:, :qw], m_ps[:, :qw])
                mu2_c = p5.tile([1, 512], F32, tag="mu2_c5", name="mu2_c5")
                nc.gpsimd.tensor_mul(mu2_c[:, :qw], mu_c[:, :qw], mu_c[:, :qw])
                var_c = p5.tile([1, 512], F32, tag="var_c5", name="var_c5")
                nc.gpsimd.tensor_sub(var_c[:, :qw], s_ps[:, :qw], mu2_c[:, :qw])
                lnv = p5.tile([1, 512], F32, tag="lnv5", name="lnv5")
                nc.scalar.activation(lnv[:, :qw], var_c[:, :qw], AF.Ln,
                                     bias=epsrow, scale=1.0)
                nc.scalar.activation(r2row[:, sl], lnv[:, :qw], AF.Exp, scale=-0.5)
            nc.vector.tensor_scalar_mul(r2row[:, 0:HALO], r2row[:, 0:HALO],
                                        consts[:, 0:1])
            nc.vector.tensor_scalar_mul(r2row[:, QE - HALO:QE],
                                        r2row[:, QE - HALO:QE], consts[:, 1:2])
            for (q0, qw) in QCH:
                r2b = ps5.tile([P, 512], F32, tag="r2b", name="r2b")
                nc.tensor.matmul(r2b[:, :qw], ones128r,
                                 r2row[:, q0:q0 + qw].bitcast(F32R),
                                 start=True, stop=True)
                nc.gpsimd.tensor_copy(R2[:, q0:q0 + qw], r2b[:, :qw])

        # ===== Phase 6: fc1 -> dwconv -> gelu -> h2 =====
        with tc.tile_pool(name="p6w", bufs=3) as p6w, \
             tc.tile_pool(name="p6h", bufs=3) as p6h, \
             tc.tile_pool(name="ps6a", bufs=2, space="PSUM") as ps6a, \
             tc.tile_pool(name="ps6b", bufs=2, space="PSUM") as ps6b:
            for m in range(NMT):
                mcol = slice(m * P, (m + 1) * P)
                dwp = p6w.tile([P, 3, 2, P], F8, tag="dwp", name="dwp")
                nc.sync.dma_start(dwp, dwp8_d[m])
                dws = p6w.tile([P, 3, P], F8, tag="dws", name="dws")
                nc.sync.dma_start(dws, dws8_d[m])
                h1p = p6h.tile([P, 14, 64], F8, tag="h1p", name="h1p")
                if m < 3:
                    nc.gpsimd.memset(h1p[:, :, 0:1], 0.0)
                    nc.gpsimd.memset(h1p[:, :, 49:64], 0.0)
                for half in range(2):
                    sl = slice(half * 336, half * 336 + 336)
                    f1 = ps6a.tile([P, 336], F32, tag="f1", name="f1")
                    for j in range(2):
                        nc.tensor.matmul(f1, wfc18_t[j][:, :, mcol],
                                         x1_8[j][:, :, sl], start=(j == 0),
                                         stop=(j == 1), perf_mode=DR)
                    dst = h1p[:, half * 7:half * 7 + 7, 1:49]
                    f1v = f1.rearrange("p (r x) -> p r x", x=48)
                    r2v = R2[:, sl].rearrange("p (r x) -> p r x", x=48)
                    if m % 3 == 2:
                        nc.gpsimd.tensor_mul(dst, f1v, r2v)
                    else:
                        nc.vector.tensor_mul(dst, f1v, r2v)
                for half in range(2):
                    cv = ps6b.tile([P, 6, 48], F32, tag="cv", name="cv")
                    r0 = 6 * half
                    for r6 in range(6):
                        R = r0 + r6
                        for dx in range(3):
                            nc.tensor.matmul(cv[:, r6, :], dwp[:, dx, :, :],
                                             h1p[:, R:R + 2, dx:dx + 48],
                                             start=(dx == 0), stop=False,
                                             perf_mode=DR)
                    for dx in range(3):
                        nc.tensor.matmul(cv, dws[:, dx, :],
                                         h1p[:, r0 + 2:r0 + 8, dx:dx + 48],
                                         start=False, stop=(dx == 2))
                    nc.scalar.activation(
                        h28[m // 2][:, m % 2, half * 288:half * 288 + 288]
                        .rearrange("p (r x) -> p r x", x=48),
                        cv, AF.Gelu, scale=2.0 ** -8)

        # ===== Phase 7: adapter + fc2 -> out =====
        with tc.tile_pool(name="ps7", bufs=2, space="PSUM") as ps7:
            a1v = a1sb.rearrange("p (a b) -> p a b", b=288)
            for ai, (q0, qw) in enumerate(ACH):
                a1_ps = ps7.tile([CA, 512], F32, tag="f2", name="a1_ps")
                for j in range(2):
                    nc.tensor.matmul(a1_ps[:, 0:288], wa18_t[j],
                                     x1_8[j][:, :, HALO + q0:HALO + q0 + qw],
                                     start=(j == 0), stop=(j == 1), perf_mode=DR)
                nc.scalar.activation(a1sb[:, ai * 288:ai * 288 + 288],
                                     a1_ps[:, 0:288], AF.Relu)
            for ai, (q0, qw) in enumerate(ACH):
                for co in range(NCT):
                    a2_ps = ps7.tile([P, 512], F32, tag="f2", name="a2_ps")
                    nc.tensor.matmul(a2_ps[:, 0:288],
                                     wa28_t[:, :, co * P:(co + 1) * P],
                                     a1v[:, ai:ai + 2, :],
                                     start=True, stop=True, perf_mode=DR)
                    nc.vector.scalar_tensor_tensor(
                        out_sb[co][:, q0:q0 + qw], a2_ps[:, 0:288], 2.0 ** -8,
                        x1_16[co][:, HALO + q0:HALO + q0 + qw],
                        op0=MUL, op1=ADD)
            for co in range(NCT):
                ccol = slice(co * P, (co + 1) * P)
                for (q0, qw) in F2CH:
                    f2 = ps7.tile([P, 512], F32, tag="f2", name="f2")
                    for mp in range(8):
                        nc.tensor.matmul(f2[:, :qw], wfc28_t[mp][:, :, ccol],
                                         h28[mp][:, :, q0:q0 + qw],
                                         start=(mp == 0), stop=(mp == 7),
                                         perf_mode=DR)
                    nc.vector.scalar_tensor_tensor(
                        out_sb[co][:, q0:q0 + qw], f2[:, :qw], 2.0 ** -4,
                        out_sb[co][:, q0:q0 + qw], op0=MUL, op1=ADD)
            for co in range(NCT):
                nc.sync.dma_start(outT_d[co * P:(co + 1) * P, :], out_sb[co])

    nc.compile()
    return nc


# ---------------- host side ----------------

_cache = {}


def _center(w):
    return w - w.mean(axis=0, keepdims=True)


def _prep_shared(inputs):
    g1 = np.asarray(inputs["g1"], np.float32)
    b1 = np.asarray(inputs["b1"], np.float32)
    g2 = np.asarray(inputs["g2"], np.float32)
    b2 = np.asarray(inputs["b2"], np.float32)
    wq = np.asarray(inputs["wq"], np.float32)
    wkv = np.asarray(inputs["wkv"], np.float32)
    wproj = np.asarray(inputs["wproj"], np.float32)
    wfc1 = np.asarray(inputs["w_fc1"], np.float32)
    wfc2 = np.asarray(inputs["w_fc2"], np.float32)
    wa1 = np.asarray(inputs["wa1"], np.float32)
    wa2 = np.asarray(inputs["wa2"], np.float32)
    dw_k = np.asarray(inputs["dw_k"], np.float32)
    for nm in ("bq", "bkv", "bproj", "b_fc1", "b_fc2", "ba1", "ba2", "dw_b"):
        assert not np.any(np.asarray(inputs[nm])), f"nonzero bias {nm} unsupported"
    assert not np.any(b1) and not np.any(b2), "nonzero LN bias unsupported"

    def pairs(w, npair_rows):
        # w [K, N] -> [K/256, 128, 2, N] with plane i = rows 128*(2j+i)
        K, N = w.shape
        return np.ascontiguousarray(
            w.reshape(K // 256, 2, 128, N).transpose(0, 2, 1, 3))

    wq_c = _center(g1[:, None] * wq) * WS
    wkv_c = _center(g1[:, None] * wkv) * WS
    wfc1_c = _center(g2[:, None] * wfc1) * WS

    k9 = dw_k[:, 0].reshape(CM, 9)  # [c, s], s = 3*dy + dx
    dwp8 = np.zeros((NMT, P, 3, 2, P), np.float32)
    dws8 = np.zeros((NMT, P, 3, P), np.float32)
    ar = np.arange(P)
    for m in range(NMT):
        blk = k9[m * P:(m + 1) * P] * WS  # [128, 9]
        for dx in range(3):
            for pl in range(2):
                dwp8[m, ar, dx, pl, ar] = blk[:, 3 * pl + dx]
            dws8[m, ar, dx, ar] = blk[:, 6 + dx]

    wa28 = np.zeros((CA, 2, C), np.float32)
    wa28[:, 0, :] = 0.5 * WS * wa2

    shared = {
        "wq8": pairs(wq_c, 2).astype(NPF8),
        "wkv8": pairs(wkv_c, 2).astype(NPF8),
        "wproj8": pairs(WS * wproj, 2).astype(NPF8),
        "wfc18": pairs(wfc1_c, 2).astype(NPF8),
        "wfc28": pairs(WS * wfc2, 8).astype(NPF8),
        "wa18": pairs(WS * wa1, 2).astype(NPF8),
        "wa28": wa28.astype(NPF8),
        "dwp8": dwp8.astype(NPF8),
        "dws8": dws8.astype(NPF8),
    }
    return shared


def run(inputs, trace=False):
    x = np.asarray(inputs["x"], np.float32)
    B, N, Cc = x.shape
    assert (B, N, Cc) == (2, NTOK, C)
    assert int(inputs["H"]) == 48 and int(inputs["W"]) == 48

    shared = _prep_shared(inputs)
    if "nc" not in _cache:
        _cache["nc"] = build()
    nc = _cache["nc"]

    in_maps = []
    for core in range(8):
        b, r = core // 4, core % 4
        roll = r * QO - HALO
        idx = (np.arange(NTOK) + roll) % NTOK
        xTc = np.ascontiguousarray(x[b].T[:, idx])
        consts = np.array([[0.0 if r == 0 else 1.0,
                            0.0 if r == 3 else 1.0, 0.0, 0.0]], np.float32)
        m = dict(shared)
        m["xT16"] = xTc.astype(np.float16)
        m["xT8"] = np.ascontiguousarray(
            xTc.reshape(2, 2, P, NTOK).transpose(0, 2, 1, 3)).astype(NPF8)
        m["consts"] = consts
        in_maps.append(m)

    res = bass_utils.run_bass_kernel_spmd(nc, in_maps, core_ids=list(range(8)),
                                          trace=trace)
    out = np.empty((B, N, C), np.float32)
    for core in range(8):
        b, r = core // 4, core % 4
        out[b, r * QO:(r + 1) * QO, :] = res.results[core]["outT"].T
    return out, res


def kernel(**inputs):
    out, _ = run(inputs, trace=False)
    return out
